# revision 7
# baseline (speedup 1.0000x reference)
"""Multi-head attention (B=1, S=4096, D=1024, H=16) on 8 trn2 NeuronCores.

Strategy: head-parallel tensor parallelism. Each core owns 2 heads:
  - Wq/Wk/Wv column-sharded by head (each core projects its 128 dims),
  - full attention for its 2 heads (flash-style streaming softmax),
  - Wo row-sharded: each core computes a partial [S, D] output,
  - host sums the 8 partials (the row-parallel unshard) after gather.

On-device layout: activations are kept transposed ([d_model, s]) so every
matmul contracts over the partition dim. The host feeds query/key/value
pre-transposed and the weight shards pre-transposed (layout prep only; all
FLOPs happen on device). Matmuls run in fp32r. The attention inner loop
keeps the PE in one tiling mode (64x128): score matmuls for the two heads
are row-packed (d_k=64 halves of the partition dim, concurrent), and the
attn@V contraction is split into two K=64 halves, cross-paired between
heads so concurrent halves hit different PSUM banks. Softmax skips
max-subtraction (scores are O(5); exp is safe in fp32) and normalizes the
64-wide attention output instead of the 4096-wide probability matrix; the
row-sum rides as a 65th ones column in the attn@V stationary operand.
"""
import numpy as np

try:
    import concourse.bass as bass  # noqa: F401
except ImportError:  # grading env fallback
    import sys
    for p in ("/opt/trn_rl_repo", "/opt/pypackages"):
        if p not in sys.path:
            sys.path.insert(0, p)

S = 4096
D_MODEL = 1024
N_CORES = 8
DH = 128              # head dims owned by one core (2 heads x 64)
D_K = 64
SB = 512              # s-block for projections
NSB = S // SB         # 8
QB = 512              # q-block for attention
NQB = S // QB         # 8
KC = 128              # k-chunk (contraction tile)
NKC = S // KC         # 32
LOOKAHEAD = 2         # scores pipeline depth (= psum "s" bufs)
ATTNV_SPLIT = True    # K=64 split attn@V (row-mode) vs K=128 M=65 full-mode
SCALE = float(D_K) ** -0.5

TRACE = False          # set by test harness for NTFF profiling
_CACHE = {}


def _build_nc():
    import concourse.bacc as bacc
    import concourse.tile as tile
    from concourse import mybir
    from concourse.masks import make_identity

    f32 = mybir.dt.float32
    f32r = mybir.dt.float32r
    Exp = mybir.ActivationFunctionType.Exp

    nc = bacc.Bacc("TRN2", target_bir_lowering=False, debug=False,
                   num_devices=N_CORES)

    xq = nc.dram_tensor("xq", [D_MODEL, S], f32r, kind="ExternalInput")
    xk = nc.dram_tensor("xk", [D_MODEL, S], f32r, kind="ExternalInput")
    xv = nc.dram_tensor("xv", [D_MODEL, S], f32r, kind="ExternalInput")
    wq = nc.dram_tensor("wq", [D_MODEL, DH], f32r, kind="ExternalInput")
    wk = nc.dram_tensor("wk", [D_MODEL, DH], f32r, kind="ExternalInput")
    wv = nc.dram_tensor("wv", [D_MODEL, DH], f32r, kind="ExternalInput")
    wo = nc.dram_tensor("wo", [DH, D_MODEL], f32r, kind="ExternalInput")
    bqv = nc.dram_tensor("bq", [DH, 1], f32, kind="ExternalInput")
    bkv = nc.dram_tensor("bk", [DH, 1], f32, kind="ExternalInput")
    bvv = nc.dram_tensor("bv", [DH, 1], f32, kind="ExternalInput")
    bov = nc.dram_tensor("bo", [1, D_MODEL], f32, kind="ExternalInput")
    out = nc.dram_tensor("out", [S, D_MODEL], f32, kind="ExternalOutput")

    with tile.TileContext(nc) as tc:
        with (
            tc.tile_pool(name="big", bufs=1) as big,
            tc.tile_pool(name="xin", bufs=6) as xin,
            tc.tile_pool(name="vtmp", bufs=2) as vtmp,
            tc.tile_pool(name="at", bufs=4) as atp,
            tc.tile_pool(name="outs", bufs=2) as outs,
            tc.tile_pool(name="small", bufs=2) as small,
            tc.tile_pool(name="ps_s", bufs=LOOKAHEAD, space="PSUM") as ps_s,
            tc.tile_pool(name="ps_o", bufs=1, space="PSUM") as ps_o,
        ):
            # ---- constants -------------------------------------------------
            ident_f = big.tile([128, 128], f32, tag="ident_f")
            nc.vector.memset(ident_f, 0.0)
            make_identity(nc, ident_f, nomemset=True)
            ident = big.tile([128, 128], f32r, tag="ident")
            nc.vector.tensor_copy(ident, ident_f)
            ones_f = big.tile([128, 1], f32, tag="ones_f")
            nc.vector.memset(ones_f, 1.0)

            # ---- weights / biases -----------------------------------------
            def w_tile(name, dram):
                t = big.tile([128, NSB, 128], f32r, tag=name)
                nc.sync.dma_start(
                    out=t, in_=dram.rearrange("(c p) m -> p c m", p=128))
                return t

            wq_sb = w_tile("wq", wq)
            wk_sb = w_tile("wk", wk)
            wv_sb = w_tile("wv", wv)
            wo_sb = big.tile([128, D_MODEL], f32r, tag="wo")
            nc.sync.dma_start(out=wo_sb, in_=wo[:, :])

            def b_tile(name, dram):
                t = big.tile([128, 1], f32, tag=name)
                nc.sync.dma_start(out=t, in_=dram[:, :])
                return t

            bq_sb = b_tile("bq", bqv)
            bk_sb = b_tile("bk", bkv)
            bv_sb = b_tile("bv", bvv)
            bo_row = big.tile([1, D_MODEL], f32, tag="bo_row")
            nc.sync.dma_start(out=bo_row, in_=bov[:, :])
            bo_b = big.tile([128, D_MODEL], f32, tag="bo_b")
            nc.gpsimd.partition_broadcast(bo_b, bo_row)

            # ---- persistent activations -----------------------------------
            qt2 = big.tile([128, S], f32r, tag="qt2")    # [dh(2 heads), s]
            kt2 = big.tile([128, S], f32r, tag="kt2")
            vnat = big.tile([128, NKC, 2, 65], f32r, tag="vnat")
            cnorm = big.tile([128, S], f32r, tag="cnorm")  # normalized attn outT

            # ---- projections (two s-blocks per pass, [128,1024] DMA chunks)
            def proj2(dst_fn, w_sb, x_dram, bias, sb2):
                s0 = sb2 * 2 * SB
                ps1 = ps_s.tile([128, SB], f32, tag="s")
                ps2 = ps_s.tile([128, SB], f32, tag="s")
                for dc in range(NSB):
                    xt = xin.tile([128, 2 * SB], f32r, tag="xt")
                    nc.sync.dma_start(
                        out=xt,
                        in_=x_dram[dc * 128:(dc + 1) * 128, s0:s0 + 2 * SB])
                    nc.tensor.matmul(ps1, w_sb[:, dc, :], xt[:, 0:SB],
                                     start=dc == 0, stop=dc == NSB - 1)
                    nc.tensor.matmul(ps2, w_sb[:, dc, :], xt[:, SB:2 * SB],
                                     start=dc == 0, stop=dc == NSB - 1)
                dst_fn(ps1, s0, bias)
                dst_fn(ps2, s0 + SB, bias)

            def qk_dst(dst):
                def f(ps, s0, bias):
                    nc.vector.tensor_scalar_add(dst[:, s0:s0 + SB], ps, bias)
                return f

            def v_dst(ps, s0, bias):
                vt_t = vtmp.tile([128, SB], f32r, tag="vt")
                nc.vector.tensor_scalar_add(vt_t, ps, bias)
                for i in range(SB // 128):
                    kc = s0 // 128 + i
                    pt = ps_s.tile([128, 128], f32r, tag="s")
                    nc.tensor.transpose(pt, vt_t[:, i * 128:(i + 1) * 128], ident)
                    for h in range(2):
                        nc.vector.tensor_copy(
                            vnat[:, kc, h, 0:64], pt[:, h * 64:(h + 1) * 64])
                        nc.vector.tensor_copy(vnat[:, kc, h, 64:65], ones_f)

            proj2(qk_dst(qt2), wq_sb, xq, bq_sb, 0)      # q-blocks 0,1 first

            # ---- attention helpers ----------------------------------------
            def mk_po():
                po0a = ps_o.tile([65, QB], f32, tag="o0a")
                po0b = ps_o.tile([65, QB], f32, tag="o0b")
                po1a = ps_o.tile([65, QB], f32, tag="o1a")
                po1b = ps_o.tile([65, QB], f32, tag="o1b")
                return po0a, po0b, po1a, po1b

            sc_tiles = {}

            def scores(kc, qsl):
                sp = ps_s.tile([128, 2 * QB], f32, tag="s")
                ksl = slice(kc * KC, (kc + 1) * KC)
                nc.tensor.matmul(sp[:, 0:QB], kt2[0:64, ksl],
                                 qt2[0:64, qsl], start=True, stop=True)
                nc.tensor.matmul(sp[:, QB:2 * QB], kt2[64:128, ksl],
                                 qt2[64:128, qsl], start=True, stop=True)
                sc_tiles[kc] = sp

            def attn_step(kc, qsl, pos, kc_ahead):
                po0a, po0b, po1a, po1b = pos
                at = atp.tile([128, 2 * QB], f32r, tag="at")
                nc.scalar.activation(at, sc_tiles.pop(kc), Exp, scale=SCALE)
                if kc_ahead is not None:
                    scores(kc_ahead, qsl)
                st, sp_ = kc == 0, kc == NKC - 1
                nc.tensor.matmul(po0a, vnat[0:64, kc, 0, :],
                                 at[0:64, 0:QB], start=st, stop=sp_)
                nc.tensor.matmul(po1b, vnat[64:128, kc, 1, :],
                                 at[64:128, QB:2 * QB], start=st, stop=sp_)
                nc.tensor.matmul(po0b, vnat[64:128, kc, 0, :],
                                 at[64:128, 0:QB], start=st, stop=sp_)
                nc.tensor.matmul(po1a, vnat[0:64, kc, 1, :],
                                 at[0:64, QB:2 * QB], start=st, stop=sp_)

            def normalize(qb, pos):
                po0a, po0b, po1a, po1b = pos
                qsl = slice(qb * QB, (qb + 1) * QB)
                for h, poa, pob in ((0, po0a, po0b), (1, po1a, po1b)):
                    po_sb = small.tile([65, QB], f32, tag="po_sb")
                    nc.vector.tensor_copy(po_sb, poa)
                    nc.vector.tensor_add(po_sb, po_sb, pob)
                    rsum = small.tile([1, QB], f32, tag="rsum")
                    nc.vector.reciprocal(rsum, po_sb[64:65, :])
                    rb = small.tile([64, QB], f32, tag="recip")
                    nc.gpsimd.partition_broadcast(rb, rsum)
                    nc.vector.tensor_mul(cnorm[h * 64:(h + 1) * 64, qsl],
                                         po_sb[0:64, :], rb)

            def outproj(qb):
                for s4 in range(QB // 128):
                    sc = qb * (QB // 128) + s4
                    csl = slice(sc * 128, (sc + 1) * 128)
                    ob = outs.tile([128, D_MODEL], f32, tag="ob")
                    for nb in range(2):
                        nsl = slice(nb * 512, (nb + 1) * 512)
                        pf = ps_s.tile([128, 512], f32, tag="s")
                        nc.tensor.matmul(pf, cnorm[:, csl], wo_sb[:, nsl],
                                         start=True, stop=True)
                        nc.vector.tensor_add(ob[:, nsl], pf, bo_b[:, nsl])
                    nc.sync.dma_start(out=out[csl, :], in_=ob)

            # ---- qb0 attention interleaved with K/V projection ------------
            qsl0 = slice(0, QB)
            pos0 = mk_po()
            KCG = NKC // (NSB // 2)      # kc chunks per proj pass (8)
            for sb2 in range(NSB // 2):
                proj2(qk_dst(kt2), wk_sb, xk, bk_sb, sb2)
                proj2(v_dst, wv_sb, xv, bv_sb, sb2)
                lo = sb2 * KCG
                scores(lo, qsl0)
                scores(lo + 1, qsl0)
                for kc in range(lo, lo + KCG):
                    ahead = kc + LOOKAHEAD
                    attn_step(kc, qsl0, pos0,
                              ahead if lo <= ahead - LOOKAHEAD < lo + KCG - LOOKAHEAD else None)
            for sb2 in range(1, NSB // 2):
                proj2(qk_dst(qt2), wq_sb, xq, bq_sb, sb2)
            normalize(0, pos0)

            # ---- remaining q-blocks ---------------------------------------
            for qb in range(1, NQB):
                qsl = slice(qb * QB, (qb + 1) * QB)
                pos = mk_po()
                scores(0, qsl)
                scores(1, qsl)
                for kc in range(NKC):
                    attn_step(kc, qsl, pos,
                              kc + LOOKAHEAD if kc + LOOKAHEAD < NKC else None)
                normalize(qb, pos)
                outproj(qb - 1)
            outproj(NQB - 1)

    nc.compile()
    return nc


def kernel(query, key, value, Wq, bq, Wk, bk, Wv, bv, Wo, bo):
    from concourse.bass_utils import run_bass_kernel_spmd

    nc = _CACHE.get("nc")
    if nc is None:
        nc = _CACHE["nc"] = _build_nc()

    f32 = np.float32
    qT = np.ascontiguousarray(np.asarray(query, f32)[0].T)
    kT = np.ascontiguousarray(np.asarray(key, f32)[0].T)
    vT = np.ascontiguousarray(np.asarray(value, f32)[0].T)
    Wq = np.asarray(Wq, f32); Wk = np.asarray(Wk, f32)
    Wv = np.asarray(Wv, f32); Wo = np.asarray(Wo, f32)
    bq = np.asarray(bq, f32); bk = np.asarray(bk, f32)
    bv = np.asarray(bv, f32); bo = np.asarray(bo, f32)

    in_maps = []
    for c in range(N_CORES):
        cs = slice(c * DH, (c + 1) * DH)
        in_maps.append({
            "xq": qT, "xk": kT, "xv": vT,
            "wq": np.ascontiguousarray(Wq[cs, :].T),
            "wk": np.ascontiguousarray(Wk[cs, :].T),
            "wv": np.ascontiguousarray(Wv[cs, :].T),
            "wo": np.ascontiguousarray(Wo[:, cs].T),
            "bq": bq[cs].reshape(DH, 1).copy(),
            "bk": bk[cs].reshape(DH, 1).copy(),
            "bv": bv[cs].reshape(DH, 1).copy(),
            "bo": (bo if c == 0 else np.zeros_like(bo)).reshape(1, D_MODEL).copy(),
        })

    res = run_bass_kernel_spmd(nc, in_maps, core_ids=list(range(N_CORES)),
                               trace=TRACE)
    _CACHE["last_results"] = res
    acc = res.results[0]["out"].astype(np.float32)
    for c in range(1, N_CORES):
        acc += res.results[c]["out"]
    return acc.reshape(1, S, D_MODEL)


# revision 8
# speedup vs baseline: 1.0740x; 1.0740x over previous
"""Multi-head attention (B=1, S=4096, D=1024, H=16) on 8 trn2 NeuronCores.

Strategy: head-parallel tensor parallelism. Each core owns 2 heads:
  - Wq/Wk/Wv column-sharded by head (each core projects its 128 dims),
  - full attention for its 2 heads (flash-style streaming softmax),
  - Wo row-sharded: each core computes a partial [S, D] output,
  - host sums the 8 partials (the row-parallel unshard) after gather.

On-device layout: activations are kept transposed ([d_model, s]) so every
matmul contracts over the partition dim. The host feeds query/key/value
pre-transposed and the weight shards pre-transposed (layout prep only; all
FLOPs happen on device). Matmuls run in fp32r. The attention inner loop
keeps the PE in one tiling mode (64x128): score matmuls for the two heads
are row-packed (d_k=64 halves of the partition dim, concurrent), and the
attn@V contraction is split into two K=64 halves, cross-paired between
heads so concurrent halves hit different PSUM banks. Softmax skips
max-subtraction (scores are O(5); exp is safe in fp32) and normalizes the
64-wide attention output instead of the 4096-wide probability matrix; the
row-sum rides as a 65th ones column in the attn@V stationary operand.
"""
import numpy as np

try:
    import concourse.bass as bass  # noqa: F401
except ImportError:  # grading env fallback
    import sys
    for p in ("/opt/trn_rl_repo", "/opt/pypackages"):
        if p not in sys.path:
            sys.path.insert(0, p)

S = 4096
D_MODEL = 1024
N_CORES = 8
DH = 128              # head dims owned by one core (2 heads x 64)
D_K = 64
SB = 512              # s-block for projections
NSB = S // SB         # 8
QB = 512              # q-block for attention
NQB = S // QB         # 8
KC = 128              # k-chunk (contraction tile)
NKC = S // KC         # 32
LOOKAHEAD = 2         # scores pipeline depth (= psum "s" bufs)
ATTNV_SPLIT = True    # K=64 split attn@V (row-mode) vs K=128 M=65 full-mode
SCALE = float(D_K) ** -0.5

TRACE = False          # set by test harness for NTFF profiling
_CACHE = {}


def _build_nc():
    import concourse.bacc as bacc
    import concourse.tile as tile
    from concourse import mybir
    from concourse.masks import make_identity

    f32 = mybir.dt.float32
    f32r = mybir.dt.float32r
    Exp = mybir.ActivationFunctionType.Exp

    nc = bacc.Bacc("TRN2", target_bir_lowering=False, debug=False,
                   num_devices=N_CORES)

    xq = nc.dram_tensor("xq", [D_MODEL, S], f32r, kind="ExternalInput")
    xk = nc.dram_tensor("xk", [D_MODEL, S], f32r, kind="ExternalInput")
    xv = nc.dram_tensor("xv", [D_MODEL, S], f32r, kind="ExternalInput")
    wq = nc.dram_tensor("wq", [D_MODEL, DH], f32r, kind="ExternalInput")
    wk = nc.dram_tensor("wk", [D_MODEL, DH], f32r, kind="ExternalInput")
    wv = nc.dram_tensor("wv", [D_MODEL, DH], f32r, kind="ExternalInput")
    wo = nc.dram_tensor("wo", [DH, D_MODEL], f32r, kind="ExternalInput")
    bqv = nc.dram_tensor("bq", [DH, 1], f32, kind="ExternalInput")
    bkv = nc.dram_tensor("bk", [DH, 1], f32, kind="ExternalInput")
    bvv = nc.dram_tensor("bv", [DH, 1], f32, kind="ExternalInput")
    bov = nc.dram_tensor("bo", [1, D_MODEL], f32, kind="ExternalInput")
    out = nc.dram_tensor("out", [S, D_MODEL], f32, kind="ExternalOutput")

    with tile.TileContext(nc) as tc:
        with (
            tc.tile_pool(name="big", bufs=1) as big,
            tc.tile_pool(name="xin", bufs=6) as xin,
            tc.tile_pool(name="vtmp", bufs=2) as vtmp,
            tc.tile_pool(name="at", bufs=4) as atp,
            tc.tile_pool(name="outs", bufs=2) as outs,
            tc.tile_pool(name="small", bufs=2) as small,
            tc.tile_pool(name="ps_s", bufs=LOOKAHEAD, space="PSUM") as ps_s,
            tc.tile_pool(name="ps_o", bufs=1, space="PSUM") as ps_o,
        ):
            # ---- constants -------------------------------------------------
            ident_f = big.tile([128, 128], f32, tag="ident_f")
            nc.vector.memset(ident_f, 0.0)
            make_identity(nc, ident_f, nomemset=True)
            ident = big.tile([128, 128], f32r, tag="ident")
            nc.vector.tensor_copy(ident, ident_f)
            ones_f = big.tile([128, 1], f32, tag="ones_f")
            nc.vector.memset(ones_f, 1.0)

            # ---- weights / biases -----------------------------------------
            def w_tile(name, dram):
                t = big.tile([128, NSB, 128], f32r, tag=name)
                nc.sync.dma_start(
                    out=t, in_=dram.rearrange("(c p) m -> p c m", p=128))
                return t

            wq_sb = w_tile("wq", wq)
            wk_sb = w_tile("wk", wk)
            wv_sb = w_tile("wv", wv)
            wo_sb = big.tile([128, D_MODEL], f32r, tag="wo")
            nc.sync.dma_start(out=wo_sb, in_=wo[:, :])

            def b_tile(name, dram):
                t = big.tile([128, 1], f32, tag=name)
                nc.sync.dma_start(out=t, in_=dram[:, :])
                return t

            bq_sb = b_tile("bq", bqv)
            bk_sb = b_tile("bk", bkv)
            bv_sb = b_tile("bv", bvv)
            bo_row = big.tile([1, D_MODEL], f32, tag="bo_row")
            nc.sync.dma_start(out=bo_row, in_=bov[:, :])
            bo_b = big.tile([128, D_MODEL], f32, tag="bo_b")
            nc.gpsimd.partition_broadcast(bo_b, bo_row)

            # ---- persistent activations -----------------------------------
            qt2 = big.tile([128, S], f32r, tag="qt2")    # [dh(2 heads), s]
            kt2 = big.tile([128, S], f32r, tag="kt2")
            vnat = big.tile([128, NKC, 2, 65], f32r, tag="vnat")
            cnorm = big.tile([128, S], f32r, tag="cnorm")  # normalized attn outT

            # ---- projections (two s-blocks per pass, [128,1024] DMA chunks)
            def proj2(dst_fn, w_sb, x_dram, bias, sb2):
                s0 = sb2 * 2 * SB
                ps1 = ps_s.tile([128, 2 * SB], f32, tag="s")
                ps2 = ps_s.tile([128, 2 * SB], f32, tag="s")
                for dc in range(NSB):
                    xt = xin.tile([128, 2 * SB], f32r, tag="xt")
                    nc.sync.dma_start(
                        out=xt,
                        in_=x_dram[dc * 128:(dc + 1) * 128, s0:s0 + 2 * SB])
                    st, sp_ = dc == 0, dc == NSB - 1
                    # K=64 row-tiled pairs; halves accumulate in different banks
                    nc.tensor.matmul(ps1[:, 0:SB], w_sb[0:64, dc, :],
                                     xt[0:64, 0:SB], start=st, stop=sp_)
                    nc.tensor.matmul(ps1[:, SB:2 * SB], w_sb[64:128, dc, :],
                                     xt[64:128, 0:SB], start=st, stop=sp_)
                    nc.tensor.matmul(ps2[:, 0:SB], w_sb[0:64, dc, :],
                                     xt[0:64, SB:2 * SB], start=st, stop=sp_)
                    nc.tensor.matmul(ps2[:, SB:2 * SB], w_sb[64:128, dc, :],
                                     xt[64:128, SB:2 * SB], start=st, stop=sp_)
                dst_fn(ps1, s0, bias)
                dst_fn(ps2, s0 + SB, bias)

            def qk_dst(dst):
                def f(ps, s0, bias):
                    nc.vector.tensor_scalar_add(dst[:, s0:s0 + SB],
                                                ps[:, 0:SB], bias)
                    nc.vector.tensor_add(dst[:, s0:s0 + SB],
                                         dst[:, s0:s0 + SB], ps[:, SB:2 * SB])
                return f

            def v_dst(ps, s0, bias):
                vt_t = vtmp.tile([128, SB], f32r, tag="vt")
                nc.vector.tensor_scalar_add(vt_t, ps[:, 0:SB], bias)
                nc.vector.tensor_add(vt_t, vt_t, ps[:, SB:2 * SB])
                for i in range(SB // 128):
                    kc = s0 // 128 + i
                    pt = ps_s.tile([128, 128], f32r, tag="s")
                    nc.tensor.transpose(pt, vt_t[:, i * 128:(i + 1) * 128], ident)
                    for h in range(2):
                        nc.vector.tensor_copy(
                            vnat[:, kc, h, 0:64], pt[:, h * 64:(h + 1) * 64])
                        nc.vector.tensor_copy(vnat[:, kc, h, 64:65], ones_f)

            proj2(qk_dst(qt2), wq_sb, xq, bq_sb, 0)      # q-blocks 0,1 first

            # ---- attention helpers ----------------------------------------
            def mk_po():
                po0a = ps_o.tile([65, QB], f32, tag="o0a")
                po0b = ps_o.tile([65, QB], f32, tag="o0b")
                po1a = ps_o.tile([65, QB], f32, tag="o1a")
                po1b = ps_o.tile([65, QB], f32, tag="o1b")
                return po0a, po0b, po1a, po1b

            sc_tiles = {}

            def scores(kc, qsl):
                sp = ps_s.tile([128, 2 * QB], f32, tag="s")
                ksl = slice(kc * KC, (kc + 1) * KC)
                nc.tensor.matmul(sp[:, 0:QB], kt2[0:64, ksl],
                                 qt2[0:64, qsl], start=True, stop=True)
                nc.tensor.matmul(sp[:, QB:2 * QB], kt2[64:128, ksl],
                                 qt2[64:128, qsl], start=True, stop=True)
                sc_tiles[kc] = sp

            def attn_step(kc, qsl, pos, kc_ahead):
                po0a, po0b, po1a, po1b = pos
                at = atp.tile([128, 2 * QB], f32r, tag="at")
                nc.scalar.activation(at, sc_tiles.pop(kc), Exp, scale=SCALE)
                if kc_ahead is not None:
                    scores(kc_ahead, qsl)
                st, sp_ = kc == 0, kc == NKC - 1
                nc.tensor.matmul(po0a, vnat[0:64, kc, 0, :],
                                 at[0:64, 0:QB], start=st, stop=sp_)
                nc.tensor.matmul(po1b, vnat[64:128, kc, 1, :],
                                 at[64:128, QB:2 * QB], start=st, stop=sp_)
                nc.tensor.matmul(po0b, vnat[64:128, kc, 0, :],
                                 at[64:128, 0:QB], start=st, stop=sp_)
                nc.tensor.matmul(po1a, vnat[0:64, kc, 1, :],
                                 at[0:64, QB:2 * QB], start=st, stop=sp_)

            def normalize(qb, pos):
                po0a, po0b, po1a, po1b = pos
                qsl = slice(qb * QB, (qb + 1) * QB)
                po_sbs = []
                for h, poa, pob in ((0, po0a, po0b), (1, po1a, po1b)):
                    po_sb = small.tile([65, QB], f32, tag="po_sb")
                    nc.vector.tensor_copy(po_sb, poa)
                    nc.vector.tensor_add(po_sb, po_sb, pob)
                    po_sbs.append(po_sb)
                for h, po_sb in enumerate(po_sbs):
                    rsum = small.tile([1, QB], f32, tag="rsum")
                    nc.vector.reciprocal(rsum, po_sb[64:65, :])
                    rb = small.tile([64, QB], f32, tag="recip")
                    nc.gpsimd.partition_broadcast(rb, rsum)
                    nc.vector.tensor_mul(cnorm[h * 64:(h + 1) * 64, qsl],
                                         po_sb[0:64, :], rb)

            def outproj(qb):
                for s4 in range(QB // 128):
                    sc = qb * (QB // 128) + s4
                    csl = slice(sc * 128, (sc + 1) * 128)
                    ob = outs.tile([128, D_MODEL], f32, tag="ob")
                    for nb in range(2):
                        nsl = slice(nb * 512, (nb + 1) * 512)
                        pf = ps_s.tile([128, 1024], f32, tag="s")
                        nc.tensor.matmul(pf[:, 0:512], cnorm[0:64, csl],
                                         wo_sb[0:64, nsl], start=True, stop=True)
                        nc.tensor.matmul(pf[:, 512:1024], cnorm[64:128, csl],
                                         wo_sb[64:128, nsl], start=True, stop=True)
                        nc.vector.tensor_add(ob[:, nsl], pf[:, 0:512], bo_b[:, nsl])
                        nc.vector.tensor_add(ob[:, nsl], ob[:, nsl], pf[:, 512:1024])
                    nc.sync.dma_start(out=out[csl, :], in_=ob)

            # ---- qb0 attention interleaved with K/V projection ------------
            qsl0 = slice(0, QB)
            pos0 = mk_po()
            KCG = NKC // (NSB // 2)      # kc chunks per proj pass (8)
            for sb2 in range(NSB // 2):
                proj2(qk_dst(kt2), wk_sb, xk, bk_sb, sb2)
                proj2(v_dst, wv_sb, xv, bv_sb, sb2)
                lo = sb2 * KCG
                scores(lo, qsl0)
                scores(lo + 1, qsl0)
                for kc in range(lo, lo + KCG):
                    ahead = kc + LOOKAHEAD
                    attn_step(kc, qsl0, pos0,
                              ahead if lo <= ahead - LOOKAHEAD < lo + KCG - LOOKAHEAD else None)
            for sb2 in range(1, NSB // 2):
                proj2(qk_dst(qt2), wq_sb, xq, bq_sb, sb2)
            normalize(0, pos0)

            # ---- remaining q-blocks ---------------------------------------
            for qb in range(1, NQB):
                qsl = slice(qb * QB, (qb + 1) * QB)
                pos = mk_po()
                scores(0, qsl)
                scores(1, qsl)
                for kc in range(NKC):
                    attn_step(kc, qsl, pos,
                              kc + LOOKAHEAD if kc + LOOKAHEAD < NKC else None)
                normalize(qb, pos)
                outproj(qb - 1)
            outproj(NQB - 1)

    nc.compile()
    return nc


def kernel(query, key, value, Wq, bq, Wk, bk, Wv, bv, Wo, bo):
    from concourse.bass_utils import run_bass_kernel_spmd

    nc = _CACHE.get("nc")
    if nc is None:
        nc = _CACHE["nc"] = _build_nc()

    f32 = np.float32
    qT = np.ascontiguousarray(np.asarray(query, f32)[0].T)
    kT = np.ascontiguousarray(np.asarray(key, f32)[0].T)
    vT = np.ascontiguousarray(np.asarray(value, f32)[0].T)
    Wq = np.asarray(Wq, f32); Wk = np.asarray(Wk, f32)
    Wv = np.asarray(Wv, f32); Wo = np.asarray(Wo, f32)
    bq = np.asarray(bq, f32); bk = np.asarray(bk, f32)
    bv = np.asarray(bv, f32); bo = np.asarray(bo, f32)

    in_maps = []
    for c in range(N_CORES):
        cs = slice(c * DH, (c + 1) * DH)
        in_maps.append({
            "xq": qT, "xk": kT, "xv": vT,
            "wq": np.ascontiguousarray(Wq[cs, :].T),
            "wk": np.ascontiguousarray(Wk[cs, :].T),
            "wv": np.ascontiguousarray(Wv[cs, :].T),
            "wo": np.ascontiguousarray(Wo[:, cs].T),
            "bq": bq[cs].reshape(DH, 1).copy(),
            "bk": bk[cs].reshape(DH, 1).copy(),
            "bv": bv[cs].reshape(DH, 1).copy(),
            "bo": (bo if c == 0 else np.zeros_like(bo)).reshape(1, D_MODEL).copy(),
        })

    res = run_bass_kernel_spmd(nc, in_maps, core_ids=list(range(N_CORES)),
                               trace=TRACE)
    _CACHE["last_results"] = res
    acc = res.results[0]["out"].astype(np.float32)
    for c in range(1, N_CORES):
        acc += res.results[c]["out"]
    return acc.reshape(1, S, D_MODEL)


# revision 9
# speedup vs baseline: 1.1902x; 1.1082x over previous
"""Multi-head attention (B=1, S=4096, D=1024, H=16) on 8 trn2 NeuronCores.

Strategy: head-parallel tensor parallelism. Each core owns 2 heads:
  - Wq/Wk/Wv column-sharded by head (each core projects its 128 dims),
  - full attention for its 2 heads (flash-style streaming softmax),
  - Wo row-sharded: each core computes a partial [S, D] output,
  - host sums the 8 partials (the row-parallel unshard) after gather.

On-device layout: activations are kept transposed ([d_model, s]) so every
matmul contracts over the partition dim. The host feeds query/key/value
pre-transposed and the weight shards pre-transposed (layout prep only; all
FLOPs happen on device). Matmuls run in fp32r. The attention inner loop
keeps the PE in one tiling mode (64x128): score matmuls for the two heads
are row-packed (d_k=64 halves of the partition dim, concurrent), and the
attn@V contraction is split into two K=64 halves, cross-paired between
heads so concurrent halves hit different PSUM banks. Softmax skips
max-subtraction (scores are O(5); exp is safe in fp32) and normalizes the
64-wide attention output instead of the 4096-wide probability matrix; the
row-sum rides as a 65th ones column in the attn@V stationary operand.
"""
import numpy as np

try:
    import concourse.bass as bass  # noqa: F401
except ImportError:  # grading env fallback
    import sys
    for p in ("/opt/trn_rl_repo", "/opt/pypackages"):
        if p not in sys.path:
            sys.path.insert(0, p)

S = 4096
D_MODEL = 1024
N_CORES = 8
DH = 128              # head dims owned by one core (2 heads x 64)
D_K = 64
SB = 512              # s-block for projections
NSB = S // SB         # 8
QB = 512              # q-block for attention
NQB = S // QB         # 8
KC = 128              # k-chunk (contraction tile)
NKC = S // KC         # 32
LOOKAHEAD = 2         # scores pipeline depth (= psum "s" bufs)
ATTNV_SPLIT = True    # K=64 split attn@V (row-mode) vs K=128 M=65 full-mode
SCALE = float(D_K) ** -0.5

TRACE = False          # set by test harness for NTFF profiling
_CACHE = {}


def _build_nc():
    import concourse.bacc as bacc
    import concourse.tile as tile
    from concourse import mybir
    from concourse.masks import make_identity

    f32 = mybir.dt.float32
    f32r = mybir.dt.float32r
    Exp = mybir.ActivationFunctionType.Exp

    nc = bacc.Bacc("TRN2", target_bir_lowering=False, debug=False,
                   num_devices=N_CORES)

    xq = nc.dram_tensor("xq", [D_MODEL, S], f32r, kind="ExternalInput")
    xk = nc.dram_tensor("xk", [D_MODEL, S], f32r, kind="ExternalInput")
    xv = nc.dram_tensor("xv", [D_MODEL, S], f32r, kind="ExternalInput")
    wq = nc.dram_tensor("wq", [D_MODEL, DH], f32r, kind="ExternalInput")
    wk = nc.dram_tensor("wk", [D_MODEL, DH], f32r, kind="ExternalInput")
    wv = nc.dram_tensor("wv", [D_MODEL, DH], f32r, kind="ExternalInput")
    wo = nc.dram_tensor("wo", [DH, D_MODEL], f32r, kind="ExternalInput")
    bqv = nc.dram_tensor("bq", [DH, 1], f32, kind="ExternalInput")
    bkv = nc.dram_tensor("bk", [DH, 1], f32, kind="ExternalInput")
    bvv = nc.dram_tensor("bv", [DH, 1], f32, kind="ExternalInput")
    bov = nc.dram_tensor("bo", [1, D_MODEL], f32, kind="ExternalInput")
    out = nc.dram_tensor("out", [S, D_MODEL], f32, kind="ExternalOutput")

    with tile.TileContext(nc) as tc:
        with (
            tc.tile_pool(name="big", bufs=1) as big,
            tc.tile_pool(name="xin", bufs=6) as xin,
            tc.tile_pool(name="vtmp", bufs=2) as vtmp,
            tc.tile_pool(name="at", bufs=4) as atp,
            tc.tile_pool(name="outs", bufs=2) as outs,
            tc.tile_pool(name="small", bufs=2) as small,
            tc.tile_pool(name="ps_s", bufs=LOOKAHEAD, space="PSUM") as ps_s,
            tc.tile_pool(name="ps_o", bufs=1, space="PSUM") as ps_o,
        ):
            # ---- constants -------------------------------------------------
            ident_f = big.tile([128, 128], f32, tag="ident_f")
            nc.vector.memset(ident_f, 0.0)
            make_identity(nc, ident_f, nomemset=True)
            ident = big.tile([128, 128], f32r, tag="ident")
            nc.vector.tensor_copy(ident, ident_f)
            ones_f = big.tile([128, 1], f32, tag="ones_f")
            nc.vector.memset(ones_f, 1.0)

            # ---- weights / biases -----------------------------------------
            def w_tile(name, dram):
                t = big.tile([128, NSB, 128], f32r, tag=name)
                nc.sync.dma_start(
                    out=t, in_=dram.rearrange("(c p) m -> p c m", p=128))
                return t

            wq_sb = w_tile("wq", wq)
            wk_sb = w_tile("wk", wk)
            wv_sb = w_tile("wv", wv)
            wo_sb = big.tile([128, D_MODEL], f32r, tag="wo")
            nc.sync.dma_start(out=wo_sb, in_=wo[:, :])

            def b_tile(name, dram):
                t = big.tile([128, 1], f32, tag=name)
                nc.sync.dma_start(out=t, in_=dram[:, :])
                return t

            bq_sb = b_tile("bq", bqv)
            bk_sb = b_tile("bk", bkv)
            bv_sb = b_tile("bv", bvv)
            bo_row = big.tile([1, D_MODEL], f32, tag="bo_row")
            nc.sync.dma_start(out=bo_row, in_=bov[:, :])
            bo_b = big.tile([128, D_MODEL], f32, tag="bo_b")
            nc.gpsimd.partition_broadcast(bo_b, bo_row)

            # ---- persistent activations -----------------------------------
            qt2 = big.tile([128, S], f32r, tag="qt2")    # [dh(2 heads), s]
            kt2 = big.tile([128, S], f32r, tag="kt2")
            vnat = big.tile([128, NKC, 2, 65], f32r, tag="vnat")
            cnorm = big.tile([128, S], f32r, tag="cnorm")  # normalized attn outT

            # ---- projections (two s-blocks per pass, [128,1024] DMA chunks)
            def proj2(dst_fn, w_sb, x_dram, bias, sb2):
                s0 = sb2 * 2 * SB
                ps1 = ps_s.tile([128, 2 * SB], f32, tag="s")
                ps2 = ps_s.tile([128, 2 * SB], f32, tag="s")
                for dc in range(NSB):
                    xt = xin.tile([128, 2 * SB], f32r, tag="xt")
                    nc.sync.dma_start(
                        out=xt,
                        in_=x_dram[dc * 128:(dc + 1) * 128, s0:s0 + 2 * SB])
                    st, sp_ = dc == 0, dc == NSB - 1
                    # K=64 row-tiled pairs; halves accumulate in different banks
                    nc.tensor.matmul(ps1[:, 0:SB], w_sb[0:64, dc, :],
                                     xt[0:64, 0:SB], start=st, stop=sp_)
                    nc.tensor.matmul(ps1[:, SB:2 * SB], w_sb[64:128, dc, :],
                                     xt[64:128, 0:SB], start=st, stop=sp_)
                    nc.tensor.matmul(ps2[:, 0:SB], w_sb[0:64, dc, :],
                                     xt[0:64, SB:2 * SB], start=st, stop=sp_)
                    nc.tensor.matmul(ps2[:, SB:2 * SB], w_sb[64:128, dc, :],
                                     xt[64:128, SB:2 * SB], start=st, stop=sp_)
                dst_fn(ps1, s0, bias)
                dst_fn(ps2, s0 + SB, bias)

            def qk_dst(dst):
                def f(ps, s0, bias):
                    nc.vector.tensor_scalar_add(dst[:, s0:s0 + SB],
                                                ps[:, 0:SB], bias)
                    nc.vector.tensor_add(dst[:, s0:s0 + SB],
                                         dst[:, s0:s0 + SB], ps[:, SB:2 * SB])
                return f

            def v_dst(ps, s0, bias):
                vt_t = vtmp.tile([128, SB], f32r, tag="vt")
                nc.vector.tensor_scalar_add(vt_t, ps[:, 0:SB], bias)
                nc.vector.tensor_add(vt_t, vt_t, ps[:, SB:2 * SB])
                for i in range(SB // 128):
                    kc = s0 // 128 + i
                    pt = ps_s.tile([128, 128], f32r, tag="s")
                    nc.tensor.transpose(pt, vt_t[:, i * 128:(i + 1) * 128], ident)
                    for h in range(2):
                        nc.vector.tensor_copy(
                            vnat[:, kc, h, 0:64], pt[:, h * 64:(h + 1) * 64])
                        nc.vector.tensor_copy(vnat[:, kc, h, 64:65], ones_f)

            proj2(qk_dst(qt2), wq_sb, xq, bq_sb, 0)      # q-blocks 0,1 first

            # ---- attention helpers ----------------------------------------
            def mk_po():
                po0a = ps_o.tile([65, QB], f32, tag="o0a")
                po0b = ps_o.tile([65, QB], f32, tag="o0b")
                po1a = ps_o.tile([65, QB], f32, tag="o1a")
                po1b = ps_o.tile([65, QB], f32, tag="o1b")
                return po0a, po0b, po1a, po1b

            sc_tiles = {}

            def scores(kc, qsl):
                sp = ps_s.tile([128, 2 * QB], f32, tag="s")
                ksl = slice(kc * KC, (kc + 1) * KC)
                nc.tensor.matmul(sp[:, 0:QB], kt2[0:64, ksl],
                                 qt2[0:64, qsl], start=True, stop=True)
                nc.tensor.matmul(sp[:, QB:2 * QB], kt2[64:128, ksl],
                                 qt2[64:128, qsl], start=True, stop=True)
                sc_tiles[kc] = sp

            def attn_step(kc, qsl, pos, kc_ahead):
                po0a, po0b, po1a, po1b = pos
                at = atp.tile([128, 2 * QB], f32r, tag="at")
                nc.scalar.activation(at, sc_tiles.pop(kc), Exp, scale=SCALE)
                if kc_ahead is not None:
                    scores(kc_ahead, qsl)
                st, sp_ = kc == 0, kc == NKC - 1
                nc.tensor.matmul(po0a, vnat[0:64, kc, 0, :],
                                 at[0:64, 0:QB], start=st, stop=sp_)
                nc.tensor.matmul(po1b, vnat[64:128, kc, 1, :],
                                 at[64:128, QB:2 * QB], start=st, stop=sp_)
                nc.tensor.matmul(po0b, vnat[64:128, kc, 0, :],
                                 at[64:128, 0:QB], start=st, stop=sp_)
                nc.tensor.matmul(po1a, vnat[0:64, kc, 1, :],
                                 at[0:64, QB:2 * QB], start=st, stop=sp_)

            def normalize(qb, pos):
                po0a, po0b, po1a, po1b = pos
                qsl = slice(qb * QB, (qb + 1) * QB)
                po_sbs = []
                for h, poa, pob in ((0, po0a, po0b), (1, po1a, po1b)):
                    po_sb = small.tile([65, QB], f32, tag="po_sb")
                    nc.vector.tensor_copy(po_sb, poa)
                    nc.vector.tensor_add(po_sb, po_sb, pob)
                    po_sbs.append(po_sb)
                for h, po_sb in enumerate(po_sbs):
                    rsum = small.tile([1, QB], f32, tag="rsum")
                    nc.vector.reciprocal(rsum, po_sb[64:65, :])
                    rb = small.tile([64, QB], f32, tag="recip")
                    nc.gpsimd.partition_broadcast(rb, rsum)
                    nc.vector.tensor_mul(cnorm[h * 64:(h + 1) * 64, qsl],
                                         po_sb[0:64, :], rb)

            def outproj(qb):
                for s4 in range(QB // 128):
                    sc = qb * (QB // 128) + s4
                    csl = slice(sc * 128, (sc + 1) * 128)
                    ob = outs.tile([128, D_MODEL], f32, tag="ob")
                    for nb in range(2):
                        nsl = slice(nb * 512, (nb + 1) * 512)
                        pf = ps_s.tile([128, 1024], f32, tag="s")
                        nc.tensor.matmul(pf[:, 0:512], cnorm[0:64, csl],
                                         wo_sb[0:64, nsl], start=True, stop=True)
                        nc.tensor.matmul(pf[:, 512:1024], cnorm[64:128, csl],
                                         wo_sb[64:128, nsl], start=True, stop=True)
                        nc.vector.tensor_add(ob[:, nsl], pf[:, 0:512], bo_b[:, nsl])
                        nc.vector.tensor_add(ob[:, nsl], ob[:, nsl], pf[:, 512:1024])
                    nc.sync.dma_start(out=out[csl, :], in_=ob)

            # ---- qb0 attention interleaved with K/V projection ------------
            qsl0 = slice(0, QB)
            pos0 = mk_po()
            KCG = NKC // (NSB // 2)      # kc chunks per proj pass (8)
            for sb2 in range(NSB // 2):
                proj2(qk_dst(kt2), wk_sb, xk, bk_sb, sb2)
                proj2(v_dst, wv_sb, xv, bv_sb, sb2)
                lo = sb2 * KCG
                scores(lo, qsl0)
                scores(lo + 1, qsl0)
                for kc in range(lo, lo + KCG):
                    ahead = kc + LOOKAHEAD
                    attn_step(kc, qsl0, pos0,
                              ahead if lo <= ahead - LOOKAHEAD < lo + KCG - LOOKAHEAD else None)
            for sb2 in range(1, NSB // 2):
                proj2(qk_dst(qt2), wq_sb, xq, bq_sb, sb2)
            normalize(0, pos0)

            # ---- remaining q-blocks ---------------------------------------
            for qb in range(1, NQB):
                qsl = slice(qb * QB, (qb + 1) * QB)
                pos = mk_po()
                scores(0, qsl)
                scores(1, qsl)
                for kc in range(NKC):
                    attn_step(kc, qsl, pos,
                              kc + LOOKAHEAD if kc + LOOKAHEAD < NKC else None)
                normalize(qb, pos)
            for qb in range(NQB):
                outproj(qb)

    nc.compile()
    return nc


def kernel(query, key, value, Wq, bq, Wk, bk, Wv, bv, Wo, bo):
    from concourse.bass_utils import run_bass_kernel_spmd

    nc = _CACHE.get("nc")
    if nc is None:
        nc = _CACHE["nc"] = _build_nc()

    f32 = np.float32
    qT = np.ascontiguousarray(np.asarray(query, f32)[0].T)
    kT = np.ascontiguousarray(np.asarray(key, f32)[0].T)
    vT = np.ascontiguousarray(np.asarray(value, f32)[0].T)
    Wq = np.asarray(Wq, f32); Wk = np.asarray(Wk, f32)
    Wv = np.asarray(Wv, f32); Wo = np.asarray(Wo, f32)
    bq = np.asarray(bq, f32); bk = np.asarray(bk, f32)
    bv = np.asarray(bv, f32); bo = np.asarray(bo, f32)

    in_maps = []
    for c in range(N_CORES):
        cs = slice(c * DH, (c + 1) * DH)
        in_maps.append({
            "xq": qT, "xk": kT, "xv": vT,
            "wq": np.ascontiguousarray(Wq[cs, :].T),
            "wk": np.ascontiguousarray(Wk[cs, :].T),
            "wv": np.ascontiguousarray(Wv[cs, :].T),
            "wo": np.ascontiguousarray(Wo[:, cs].T),
            "bq": bq[cs].reshape(DH, 1).copy(),
            "bk": bk[cs].reshape(DH, 1).copy(),
            "bv": bv[cs].reshape(DH, 1).copy(),
            "bo": (bo if c == 0 else np.zeros_like(bo)).reshape(1, D_MODEL).copy(),
        })

    res = run_bass_kernel_spmd(nc, in_maps, core_ids=list(range(N_CORES)),
                               trace=TRACE)
    _CACHE["last_results"] = res
    acc = res.results[0]["out"].astype(np.float32)
    for c in range(1, N_CORES):
        acc += res.results[c]["out"]
    return acc.reshape(1, S, D_MODEL)
